# revision 17
# baseline (speedup 1.0000x reference)
"""Trainium2 Bass kernel for sliding-window unfold (im2col).

reference:  out = x[:, idx, :]  with idx[w, f] = w + f
  x:   [128, 4096, 4]  f32
  out: [128, 4065, 32, 4]  f32

out[b, w] (= 128 floats = 512 B) is the contiguous slice
x[b].flat[4w : 4w + 128]; HBM write bandwidth is the roofline.

Measured on TRN2 (trace analysis across many runs):
  - dma_starts spanning EXACTLY 128 SBUF partitions spray across all
    16 SDMA engines; DRAM->DRAM DMAs land on ONE engine (never).
  - queues dispatch ~16 ns/descriptor (128-row DMA ~2.05 us of queue
    time); each DMA_DIRECT2D costs ~0.7-1.7 us on the issuing engine.
  - ONE dense store queue writes sequentially at ~26.6 GB/s/engine
    (~591 ns per 15,872 B packet); TWO heavy interleaved store streams
    stretch packets to ~800 ns (~20 GB/s) - HBM locality matters.
    So the bulk stream must stay a single dense queue.
  - a single-queue-only store stream develops a ~20% straggler engine;
    keeping a little store traffic on the second queue historically
    removes it.
  - DVE expand cost is nonlinear: 31-window copy 5.1 us, 16-window
    1.22 us.  Two half-expands on DVE = ~2.4 us/batch.

Structure (pure data parallel, 16 batches/core on 8 cores):
  - out_bulk [16, 3968, 32, 4]: windows 0..3967, written ONLY by the
    gpsimd/SWDGE queue as 16 dense back-to-back batch stores
    (15.5 KB/partition rows, sequential DRAM addresses).
  - out_tail [16, 128, 32, 4]: windows 3937..4064 (31 overlap vs bulk
    kept to stay on the 128-partition fast path; no byte in out_bulk
    is ever touched by the tail stream - different tensors), written
    ONLY by the scalar HWDGE queue, fully in parallel mid-stream.
  - kernel() assembles: out[:, :3968] = bulk, out[:, 3968:] = tail[:, 31:].
  Per batch: X load [128, 248] (sync queue carries ONLY the 16 X
  loads, so the last expands never starve the bulk stream); expand
  X -> Y[128, 3968] as two DVE half-copies; bulk store.  Tail tiles
  [128, 128] load on the scalar queue, tail stores behind them on the
  same queue - the whole tail pipeline is independent of the bulk one.
"""

import numpy as np

from concourse import bacc, mybir, tile
from concourse.bass_utils import run_bass_kernel_spmd

N_CORES = 8
B_FULL = 128
B = B_FULL // N_CORES  # 16 batches per core
S = 4096
C = 4
F = 32
W = S - F + 1    # 4065
FL = F * C       # 128 floats per window
XB = S * C       # 16384 floats per batch of x
WPP = 31         # windows per partition in the bulk store
NBULK = 128 * WPP          # 3968 bulk windows per batch
NTAIL = W - NBULK          # 97 real tail windows (stored as 128)
YROW = WPP * FL            # 3968 floats per partition row
XROW = (WPP - 1) * C + FL  # 248 floats of x per partition
WSPLIT = 16                # windows per DVE half-expand
OBB = NBULK * FL           # floats per batch of out_bulk
OBT = 128 * FL             # floats per batch of out_tail

_cache = {}


def build_nc():
    nc = bacc.Bacc("TRN2", target_bir_lowering=False)
    x = nc.dram_tensor("x", [B, S, C], mybir.dt.float32, kind="ExternalInput")
    out_bulk = nc.dram_tensor("out_bulk", [B, NBULK, F, C], mybir.dt.float32,
                              kind="ExternalOutput")
    out_tail = nc.dram_tensor("out_tail", [B, 128, F, C], mybir.dt.float32,
                              kind="ExternalOutput")

    with tile.TileContext(nc) as tc:
        with (
            tc.tile_pool(name="xp", bufs=8) as xp,
            tc.tile_pool(name="yp", bufs=12) as yp,
            tc.tile_pool(name="tp", bufs=8) as tp,
        ):
            # -- X loads on the sync queue, compressed first.
            Xs = []
            for b in range(B):
                X = xp.tile([128, XROW], mybir.dt.float32)
                src = x[:].copy()
                src.ap = mybir.VecI64Pair([[WPP * C, 128], [1, XROW]])
                src.offset = b * XB
                nc.sync.dma_start(out=X[:, :], in_=src)
                Xs.append(X)

            # -- tail tile loads on the scalar queue, two batches per
            #    DMA (fewer DMAs -> fewer shared-semaphore users, and
            #    the tail pipeline finishes early).
            TB2s = []
            for k in range(B // 2):
                TB = tp.tile([128, 2 * FL], mybir.dt.float32)
                srcT = x[:].copy()
                srcT.ap = mybir.VecI64Pair([[C, 128], [XB, 2], [1, FL]])
                srcT.offset = 2 * k * XB + (NBULK - 31) * C
                nc.scalar.dma_start(out=TB[:, :], in_=srcT)
                TB2s.append(TB)

            # -- tail stores on the scalar queue (own tensor, fully
            #    parallel to the bulk stream), two batches per DMA.
            for k in range(B // 2):
                dstT = out_tail[:].copy()
                dstT.ap = mybir.VecI64Pair([[FL, 128], [OBT, 2], [1, FL]])
                dstT.offset = 2 * k * OBT
                srcS = TB2s[k][:].copy()
                srcS.ap = mybir.VecI64Pair([[2 * FL, 128], [FL, 2], [1, FL]])
                srcS.offset = 0
                nc.scalar.dma_start(out=dstT, in_=srcS)

            # -- expand (2x DVE halves) + ONE dense bulk store stream.
            for b in range(B):
                X = Xs[b]
                Y = yp.tile([128, YROW], mybir.dt.float32)

                srcA = X[:].copy()
                srcA.ap = mybir.VecI64Pair([[XROW, 128], [C, WSPLIT], [1, FL]])
                srcA.offset = 0
                dstA = Y[:].copy()
                dstA.ap = mybir.VecI64Pair([[YROW, 128], [FL, WSPLIT], [1, FL]])
                dstA.offset = 0
                nc.vector.tensor_copy(out=dstA, in_=srcA)

                srcB = X[:].copy()
                srcB.ap = mybir.VecI64Pair([[XROW, 128], [C, WPP - WSPLIT], [1, FL]])
                srcB.offset = WSPLIT * C
                dstB = Y[:].copy()
                dstB.ap = mybir.VecI64Pair([[YROW, 128], [FL, WPP - WSPLIT], [1, FL]])
                dstB.offset = WSPLIT * FL
                nc.vector.tensor_copy(out=dstB, in_=srcB)

                dst3 = out_bulk[:].copy()
                dst3.ap = mybir.VecI64Pair([[YROW, 128], [1, YROW]])
                dst3.offset = b * OBB
                nc.gpsimd.dma_start(out=dst3, in_=Y[:, :])

    nc.finalize()
    return nc


def run_sharded(x: np.ndarray, trace: bool = False):
    """Shard batch across 8 cores, run, gather. Returns (out, raw results)."""
    if "nc" not in _cache:
        _cache["nc"] = build_nc()
    nc = _cache["nc"]

    x = np.ascontiguousarray(x, dtype=np.float32)
    in_maps = [{"x": x[i * B : (i + 1) * B]} for i in range(N_CORES)]
    res = run_bass_kernel_spmd(nc, in_maps, list(range(N_CORES)), trace=trace)
    out = np.empty((B_FULL, W, F, C), dtype=np.float32)
    for i in range(N_CORES):
        lo = i * B
        out[lo : lo + B, :NBULK] = res.results[i]["out_bulk"]
        out[lo : lo + B, NBULK:] = res.results[i]["out_tail"][:, 31:]
    return out, res


def kernel(x: np.ndarray) -> np.ndarray:
    out, _ = run_sharded(x, trace=False)
    return out


# revision 20
# speedup vs baseline: 1.0772x; 1.0772x over previous
"""Trainium2 Bass kernel for sliding-window unfold (im2col).

reference:  out = x[:, idx, :]  with idx[w, f] = w + f
  x:   [128, 4096, 4]  f32
  out: [128, 4065, 32, 4]  f32

out[b, w] (= 128 floats = 512 B) is the contiguous slice
x[b].flat[4w : 4w + 128]; HBM write bandwidth is the roofline.

Measured on TRN2 (trace analysis across many runs):
  - dma_starts spanning EXACTLY 128 SBUF partitions spray across all
    16 SDMA engines; DRAM->DRAM DMAs land on ONE engine (never).
  - queues dispatch ~16 ns/descriptor (128-row DMA ~2.05 us of queue
    time); each DMA_DIRECT2D costs ~0.7-1.7 us on the issuing engine.
  - ONE dense store queue writes sequentially at ~26.6 GB/s/engine
    (~591 ns per 15,872 B packet); TWO heavy interleaved store streams
    stretch packets to ~800 ns (~20 GB/s) - HBM locality matters.
    So the bulk stream must stay a single dense queue.
  - a single-queue-only store stream develops a ~20% straggler engine;
    keeping a little store traffic on the second queue historically
    removes it.
  - DVE expand cost is nonlinear: 31-window copy 5.1 us, 16-window
    1.22 us.  Two half-expands on DVE = ~2.4 us/batch.

Structure (pure data parallel, 16 batches/core on 8 cores):
  - out_bulk [16, 3968, 32, 4]: windows 0..3967, written ONLY by the
    gpsimd/SWDGE queue as 16 dense back-to-back batch stores
    (15.5 KB/partition rows, sequential DRAM addresses).
  - out_tail [16, 128, 32, 4]: windows 3937..4064 (31 overlap vs bulk
    kept to stay on the 128-partition fast path; no byte in out_bulk
    is ever touched by the tail stream - different tensors), written
    ONLY by the scalar HWDGE queue, fully in parallel mid-stream.
  - kernel() assembles: out[:, :3968] = bulk, out[:, 3968:] = tail[:, 31:].
  Per batch: X load [128, 248] (sync queue carries ONLY the 16 X
  loads, so the last expands never starve the bulk stream); expand
  X -> Y[128, 3968] as two DVE half-copies; bulk store.  Tail tiles
  [128, 128] load on the scalar queue, tail stores behind them on the
  same queue - the whole tail pipeline is independent of the bulk one.
"""

import numpy as np

from concourse import bacc, mybir, tile
from concourse.bass_utils import run_bass_kernel_spmd

N_CORES = 8
B_FULL = 128
B = B_FULL // N_CORES  # 16 batches per core
S = 4096
C = 4
F = 32
W = S - F + 1    # 4065
FL = F * C       # 128 floats per window
XB = S * C       # 16384 floats per batch of x
WPP = 31         # windows per partition in the bulk store
NBULK = 128 * WPP          # 3968 bulk windows per batch
NTAIL = W - NBULK          # 97 real tail windows (stored as 128)
YROW = WPP * FL            # 3968 floats per partition row
XROW = (WPP - 1) * C + FL  # 248 floats of x per partition
WSPLIT = 16                # windows per DVE half-expand
OBB = NBULK * FL           # floats per batch of out_bulk
OBT = 128 * FL             # floats per batch of out_tail

_cache = {}


def build_nc():
    nc = bacc.Bacc("TRN2", target_bir_lowering=False)
    x = nc.dram_tensor("x", [B, S, C], mybir.dt.float32, kind="ExternalInput")
    out_bulk = nc.dram_tensor("out_bulk", [B, NBULK, F, C], mybir.dt.float32,
                              kind="ExternalOutput")
    out_tail = nc.dram_tensor("out_tail", [B, 128, F, C], mybir.dt.float32,
                              kind="ExternalOutput")

    with tile.TileContext(nc) as tc:
        with (
            tc.tile_pool(name="xp", bufs=8) as xp,
            tc.tile_pool(name="yp", bufs=12) as yp,
            tc.tile_pool(name="tp", bufs=16) as tp,
        ):
            # -- X loads on the sync queue, compressed first.
            Xs = []
            for b in range(B):
                X = xp.tile([128, XROW], mybir.dt.float32)
                src = x[:].copy()
                src.ap = mybir.VecI64Pair([[WPP * C, 128], [1, XROW]])
                src.offset = b * XB
                nc.sync.dma_start(out=X[:, :], in_=src)
                Xs.append(X)

            # -- tail pipeline, one batch per DMA (the contiguous store
            #    dst aggregates into 4 KB packets; pair-merged 512 B
            #    packets measurably slow the bulk stream by ~20%).
            #    Tails 0..11 ride the scalar queue; 12..15 ride sync
            #    behind the X loads, so ALL tail DMAs complete by
            #    ~60 us - their completions share semaphores with the
            #    X loads and can otherwise stall late expands.
            NT_SC = 12

            TBs = []
            for b in range(B):
                TB = tp.tile([128, FL], mybir.dt.float32)
                srcT = x[:].copy()
                srcT.ap = mybir.VecI64Pair([[C, 128], [1, FL]])
                srcT.offset = b * XB + (NBULK - 31) * C
                (nc.scalar if b < NT_SC else nc.sync).dma_start(out=TB[:, :], in_=srcT)
                TBs.append(TB)

            for b in range(B):
                dstT = out_tail[:].copy()
                dstT.ap = mybir.VecI64Pair([[FL, 128], [1, FL]])
                dstT.offset = b * OBT
                (nc.scalar if b < NT_SC else nc.sync).dma_start(out=dstT, in_=TBs[b][:, :])

            # -- expand (2x DVE halves) + ONE dense bulk store stream.
            for b in range(B):
                X = Xs[b]
                Y = yp.tile([128, YROW], mybir.dt.float32)

                srcA = X[:].copy()
                srcA.ap = mybir.VecI64Pair([[XROW, 128], [C, WSPLIT], [1, FL]])
                srcA.offset = 0
                dstA = Y[:].copy()
                dstA.ap = mybir.VecI64Pair([[YROW, 128], [FL, WSPLIT], [1, FL]])
                dstA.offset = 0
                nc.vector.tensor_copy(out=dstA, in_=srcA)

                srcB = X[:].copy()
                srcB.ap = mybir.VecI64Pair([[XROW, 128], [C, WPP - WSPLIT], [1, FL]])
                srcB.offset = WSPLIT * C
                dstB = Y[:].copy()
                dstB.ap = mybir.VecI64Pair([[YROW, 128], [FL, WPP - WSPLIT], [1, FL]])
                dstB.offset = WSPLIT * FL
                nc.vector.tensor_copy(out=dstB, in_=srcB)

                dst3 = out_bulk[:].copy()
                dst3.ap = mybir.VecI64Pair([[YROW, 128], [1, YROW]])
                dst3.offset = b * OBB
                nc.gpsimd.dma_start(out=dst3, in_=Y[:, :])

    nc.finalize()
    return nc


def run_sharded(x: np.ndarray, trace: bool = False):
    """Shard batch across 8 cores, run, gather. Returns (out, raw results)."""
    if "nc" not in _cache:
        _cache["nc"] = build_nc()
    nc = _cache["nc"]

    x = np.ascontiguousarray(x, dtype=np.float32)
    in_maps = [{"x": x[i * B : (i + 1) * B]} for i in range(N_CORES)]
    res = run_bass_kernel_spmd(nc, in_maps, list(range(N_CORES)), trace=trace)
    out = np.empty((B_FULL, W, F, C), dtype=np.float32)
    for i in range(N_CORES):
        lo = i * B
        out[lo : lo + B, :NBULK] = res.results[i]["out_bulk"]
        out[lo : lo + B, NBULK:] = res.results[i]["out_tail"][:, 31:]
    return out, res


def kernel(x: np.ndarray) -> np.ndarray:
    out, _ = run_sharded(x, trace=False)
    return out
